# revision 40
# baseline (speedup 1.0000x reference)
"""Trainium2 Bass kernel for nn_BaselineBlock_SCA_Modulated (v2, fp8 conv).

Sharding: 8 cores = 2 batch x 4 D-slabs of 16 planes. Halo planes staged
host-side (zero at global D edges) so all cores run one SPMD program.

Main ideas vs v1:
- The fused pw1+3x3x3-depthwise conv runs as 10 fp8e4m3 DoubleRow matmuls
  per 512-position chunk (each contracting 2x128 rows at 0.5 cyc/row),
  reading R-tiles that stack two consecutive padded planes (64ch + 64ch)
  in the partition dim; the j-dim of DoubleRow pairs two (kh,kw) taps with
  an even element-stride delta. The pw1-bias boundary correction rides a
  spare j-slot against an indicator region stored beside each plane.
- LN statistics via one squared pass + two DVE reduces; rsqrt computed on
  DVE with the bit-trick + one Newton step (no act-table switching: the
  Act engine only ever uses the gelu table).
- All layout flips are DMA transposes (SBUF<->SBUF bf16); padding+fp8
  conversion of conv input happens on gpsimd (Pool).
- xg (post-gelu conv output) stays SBUF-resident in fp8; w3 is a plain
  fp8 matmul; pass-2 psum evacuations are split between DVE (ISA affine)
  and Act (identity-matmul accumulate + scaled copy) to balance engines.
"""
import numpy as np
import ml_dtypes

C, DW, SD = 64, 128, 512
D, H, W = 64, 64, 64
NPL = 16              # output planes per core
NHALO = NPL + 2
PW = 66               # padded row width
PLA = PW * PW         # padded plane area = 4356
REGB = PLA + 4        # region base (4360, even)
REGS = 544            # per-pattern spacing
RW = REGB + 3 * REGS  # R tile width = 5992
HWC = H * W
EPS = 1e-6
MAGIC = 0x5F3759DF
bf = ml_dtypes.bfloat16
f8 = ml_dtypes.float8_e4m3fn

# which pass-2 chunk slots evacuate via DVE ISA-affine (rest via Act)
Y_DVE = (0, 2)
O_DVE = ()

_CACHE = {}


def _build():
    import concourse.bacc as bacc
    import concourse.mybir as mybir
    import concourse.tile as tile
    from concourse.ap import AP as BassAP
    from concourse.mybir import ActivationFunctionType as AF, AluOpType as ALU

    BF = mybir.dt.bfloat16
    F32 = mybir.dt.float32
    F8 = mybir.dt.float8e4
    I32 = mybir.dt.int32
    AX = mybir.AxisListType
    DR = mybir.MatmulPerfMode.DoubleRow

    nc = bacc.Bacc("TRN2", target_bir_lowering=False, debug=False, num_devices=8)

    dram = {}
    def din(name, shape, dt):
        dram[name] = nc.dram_tensor(name, shape, dt, kind="ExternalInput")
        return dram[name]

    import os
    inp_t = din("inp_t", [NHALO, C, HWC], BF)
    inp_b = din("inp_b", [NPL, C, HWC], BF)
    wa_i = din("wa", [128, 4, 2, 128], F8)
    wa4_i = din("wa4", [128, NPL, 2, 128], F8)
    wb_i = din("wb", [128, 4, 2, 128], F8)
    wb4_i = din("wb4", [128, 2, 128], F8)
    reg_i = din("reg", [9, 3 * REGS], F8)
    sd8_i = din("sd8", [128, 1], F32)
    modb_i = din("modb2", [128, 1], F32)
    w3tp_i = din("w3tp", [128, 64], BF)
    identA_i = din("identA", [65, 64], BF)
    identB_i = din("identB", [65, 64], BF)
    beta3_i = din("beta3", [64, 1], F32)
    b3b_i = din("b3b", [64, 1], F32)
    w4T_i = din("w4T", [64, 128], BF)
    b4_i = din("b4", [128, 1], F32)
    w5gT_i = din("w5gT", [128, 64], BF)
    b5g_i = din("b5g", [64, 1], F32)
    scawT_i = din("scawT", [128, 128], BF)
    scab_i = din("scab", [128, 1], F32)
    out_d = nc.dram_tensor("out", [NPL, C, HWC], BF, kind="ExternalOutput")
    import os
    DBG = bool(int(os.environ.get("NKDEBUG", "0")))
    if DBG:
        dbg_xg = nc.dram_tensor("dbg_xg", [128, NPL * HWC], F8,
                                kind="ExternalOutput")
        dbg_xln = nc.dram_tensor("dbg_xln", [64, HWC], BF,
                                 kind="ExternalOutput")
        dbg_R = nc.dram_tensor("dbg_R", [128, RW], F8, kind="ExternalOutput")
        dbg_y = nc.dram_tensor("dbg_y", [65, HWC], BF, kind="ExternalOutput")
        dbg_ps = nc.dram_tensor("dbg_ps", [128, 512], F32,
                                kind="ExternalOutput")

    cc_a = nc.dram_tensor("cc_a", [128, 1], F32)
    cc_b = nc.dram_tensor("cc_b", [4, 128], F32)
    ones_i = din("ones", [1, HWC], BF)

    from contextlib import ExitStack
    LIN = bool(int(os.environ.get("NKLIN", "0")))
    with tile.TileContext(nc, linearize=LIN) as tc, ExitStack() as stk:
        cpool = stk.enter_context(tc.tile_pool(name="const", bufs=1))
        rpool = stk.enter_context(tc.tile_pool(name="ring", bufs=1))
        wpool = stk.enter_context(tc.tile_pool(name="work", bufs=2))
        psA = stk.enter_context(tc.tile_pool(name="psA", bufs=2, space="PSUM"))
        psB = stk.enter_context(tc.tile_pool(name="psB", bufs=2, space="PSUM"))

        def const(name, shape, dt):
            t = cpool.tile(shape, dt, tag=name, name=name)
            nc.sync.dma_start(t[:], dram[name][:])
            return t

        # consts needed early (pass-1 conv); the pass-2 consts are loaded
        # after the pass-1 loop is issued so the first plane transposes
        # aren't stuck behind them in the in-order SP/HWDGE queue
        wa = const("wa", [128, 4, 2, 128], F8)
        wa4 = const("wa4", [128, NPL, 2, 128], F8)
        wb = const("wb", [128, 4, 2, 128], F8)
        wb4 = const("wb4", [128, 2, 128], F8)
        sd8 = const("sd8", [128, 1], F32)
        modb2 = const("modb2", [128, 1], F32)

        xg = cpool.tile([128, NPL * HWC], F8, tag="xg", name="xg")
        pools = cpool.tile([128, NPL * 4], F32, tag="pools")
        w3g8 = cpool.tile([128, 64], F8, tag="w3g8")

        NS = 4
        Rs = [rpool.tile([128, RW], F8, tag=f"R{i}", name=f"R{i}")
              for i in range(NS)]
        for i in range(NS):
            eng = nc.gpsimd if i % 2 == 0 else nc.vector
            eng.memset(Rs[i][:, PLA:RW], 0.0)         # slack + region cols
            eng.memset(Rs[i][:, 0:PW], 0.0)           # row 0
            eng.memset(Rs[i][:, 65 * PW:PLA], 0.0)    # row 65
            v = Rs[i][:, 0:1]
            from concourse.ap import AP as _AP
            eng.memset(_AP(v.tensor, v.offset, [list(v.ap[0]), [PW, 66], [1, 1]]), 0.0)
            eng.memset(_AP(v.tensor, v.offset + 65, [list(v.ap[0]), [PW, 66], [1, 1]]), 0.0)
            nc.sync.dma_start(Rs[i][0:9, REGB:RW], reg_i[:])
        ys = [rpool.tile([65, HWC], BF, tag=f"y{i}", name=f"y{i}")
              for i in range(3)]
        ifps = [rpool.tile([65, HWC], BF, tag=f"ifp{i}", name=f"ifp{i}")
                for i in range(2)]
        for i in range(3):
            nc.sync.dma_start(ys[i][64:65, :], ones_i[:])
        for i in range(2):
            nc.sync.dma_start(ifps[i][64:65, :], ones_i[:])


        # pass-2 scratch carved from the R slabs (dead after pass 1):
        # outb ring in cols 0:2048 (bf16), xg2 ring in cols 2048:2560
        Rv = [Rs[i][:].bitcast(BF) for i in range(NS)]

        def rsqrt_dve(out_f32, a_f32, shape, scale):
            """out = scale / sqrt(a), one Newton step. Tiles [128, n] f32."""
            n = shape[1]
            y0i = wpool.tile(shape, I32, tag="y0i")
            nc.vector.tensor_scalar(y0i[:], a_f32.bitcast(I32), 1, None,
                                    op0=ALU.arith_shift_right)
            nc.vector.tensor_scalar(y0i[:], y0i[:], MAGIC, -1,
                                    op0=ALU.subtract, op1=ALU.mult)
            y0 = y0i[:].bitcast(F32)
            y2 = wpool.tile(shape, F32, tag="y2")
            nc.vector.tensor_mul(y2[:], y0, y0)
            nc.vector.tensor_mul(y2[:], y2[:], a_f32)
            nc.vector.tensor_scalar(y2[:], y2[:], -0.5, 1.5,
                                    op0=ALU.mult, op1=ALU.add)
            nc.vector.scalar_tensor_tensor(out_f32, y0, scale, y2[:],
                                           op0=ALU.mult, op1=ALU.mult)

        def interleave2(dst, src, n):
            """dst[128, 2n] <- src[128, n] interleaved (each value twice)."""
            v = dst[:, 0:2 * n:2]
            nc.vector.tensor_copy(v, src[:])
            v2 = dst[:, 1:2 * n:2]
            nc.vector.tensor_copy(v2, src[:])

        def pair4(t, n):
            """[128, n*64] tile viewed as [128, n, 32, 2] (G, c', b)."""
            v = t[:, 0:1]
            return BassAP(v.tensor, v.offset,
                          [list(v.ap[0]), [64, n], [2, 32], [1, 2]])

        def bcast4(t2, n):
            """interleaved [128, 2n] viewed as [128, n, 32, 2] broadcast."""
            v = t2[:, 0:1]
            return BassAP(v.tensor, v.offset,
                          [list(v.ap[0]), [2, n], [0, 32], [1, 2]])

        GS = 30

        def split_sq(xT, xln):
            # squares: groups [0,GS) on DVE, [GS,32) on Pool, into xln wide
            v = xln[:, 0:1]
            dve_v = BassAP(v.tensor, v.offset, [list(v.ap[0]), [128, GS], [1, 64]])
            plv = BassAP(v.tensor, v.offset + 128 * GS,
                         [list(v.ap[0]), [128, 32 - GS], [1, 64]])
            nc.vector.tensor_tensor(dve_v, xT[:, 0:GS], xT[:, 0:GS],
                                    op=ALU.mult)
            nc.gpsimd.tensor_tensor(plv, xT[:, GS:32], xT[:, GS:32],
                                    op=ALU.mult)

        def ln_stats(xT, sq_tile, r2, ng):
            """xT [128, ng, 64] bf16 -> r2/mr2 [128, 2*ng] interleaved.

            TensorReduce runs at 1x on DVE regardless of dtype, so fold the
            64-wide groups down to 4 with packed-bf16 tensor_tensor adds (2x
            mode) and only TensorReduce the 4-wide stub. The x-tree folds
            into a scratch tile; the q-tree folds the squares in place
            (their region is dead after the stats)."""
            tr = wpool.tile([128, ng, 32], BF, tag="tr")

            def sx(off, n):
                return BassAP(xT.tensor, xT.offset + off,
                              [list(xT.ap[0]), [64, ng], [1, n]])

            def sq(off, n):
                return BassAP(sq_tile.tensor, sq_tile.offset + off,
                              [list(sq_tile.ap[0]), [64, ng], [1, n]])

            def st(off, n):
                v = tr[:, 0:1]
                return BassAP(v.tensor, v.offset + off,
                              [list(v.ap[0]), [32, ng], [1, n]])

            # interleave the two independent fold chains so the dependent-op
            # ack gaps of one hide under the other's processing
            nc.vector.tensor_tensor(st(0, 32), sx(0, 32), sx(32, 32),
                                    op=ALU.add)
            nc.vector.tensor_tensor(sq(0, 32), sq(0, 32), sq(32, 32),
                                    op=ALU.add)
            nc.vector.tensor_tensor(st(0, 16), st(0, 16), st(16, 16),
                                    op=ALU.add)
            nc.vector.tensor_tensor(sq(0, 16), sq(0, 16), sq(16, 16),
                                    op=ALU.add)
            nc.vector.tensor_tensor(st(0, 8), st(0, 8), st(8, 8), op=ALU.add)
            nc.vector.tensor_tensor(sq(0, 8), sq(0, 8), sq(8, 8), op=ALU.add)
            nc.vector.tensor_tensor(st(0, 4), st(0, 4), st(4, 4), op=ALU.add)
            nc.vector.tensor_tensor(sq(0, 4), sq(0, 4), sq(4, 4), op=ALU.add)
            ms = wpool.tile([128, ng], F32, tag="ms")
            qs = wpool.tile([128, ng], F32, tag="qs")
            nc.vector.tensor_reduce(ms[:], st(0, 4), axis=AX.X, op=ALU.add)
            nc.vector.tensor_reduce(qs[:], sq(0, 4), axis=AX.X, op=ALU.add)
            t1 = wpool.tile([128, ng], F32, tag="t1v")
            nc.vector.tensor_mul(t1[:], ms[:], ms[:])
            av = wpool.tile([128, ng], F32, tag="av")
            nc.vector.scalar_tensor_tensor(av[:], t1[:], -1.0 / 64.0, qs[:],
                                           op0=ALU.mult, op1=ALU.add)
            # rsqrt bit-trick + one Newton step; rv and mr land in one f32
            # tile so a single strided copy emits the interleaved bf16 pairs
            rv2 = wpool.tile([128, 2 * ng], F32, tag="rv2")
            rv = rv2[:, 0:ng]
            rsqrt_dve(rv, av[:], [128, ng], float(np.sqrt(63.0)))
            nc.vector.scalar_tensor_tensor(rv2[:, ng:2 * ng], ms[:],
                                           1.0 / 64.0, rv,
                                           op0=ALU.mult, op1=ALU.mult)
            vo = r2[:, 0:1]
            vi = rv2[:, 0:1]
            nc.vector.tensor_copy(
                BassAP(vo.tensor, vo.offset,
                       [list(vo.ap[0]), [2 * ng, 2], [2, ng], [1, 2]]),
                BassAP(vi.tensor, vi.offset,
                       [list(vi.ap[0]), [ng, 2], [1, ng], [0, 2]]))

        # ---------------- PASS 1 ----------------
        def wideview(t, inner=64):
            # [128, 32, inner] strided view of the UPPER half of each
            # 128-wide group of a [128, 32, 128] tile (squares live there so
            # every element of the tile is written each plane - no warm
            # memsets needed before the full-tile transpose reads it)
            v = t[:, 0:1]
            return BassAP(v.tensor, v.offset + 64,
                          [list(v.ap[0]), [128, 32], [1, inner]])

        def pair4w(t):
            v = t[:, 0:1]
            return BassAP(v.tensor, v.offset,
                          [list(v.ap[0]), [128, 32], [2, 32], [1, 2]])

        def norm_split(xln, xT, r2, mr2, engs=None):
            if engs is None:
                engs = ((nc.vector, 0, 32),)
            v = xln[:, 0:1]
            xv = xT[:, 0:1]
            rv_ = r2[:, 0:1]
            mv_ = mr2[:, 0:1]
            for eng, g0, g1 in engs:
                n = g1 - g0
                wo = BassAP(v.tensor, v.offset + 64 * g0,
                            [list(v.ap[0]), [64, n], [2, 32], [1, 2]])
                xi = BassAP(xv.tensor, xv.offset + 64 * g0,
                            [list(xv.ap[0]), [64, n], [2, 32], [1, 2]])
                rb = BassAP(rv_.tensor, rv_.offset + 2 * g0,
                            [list(rv_.ap[0]), [2, n], [0, 32], [1, 2]])
                mb = BassAP(mv_.tensor, mv_.offset + 2 * g0,
                            [list(mv_.ap[0]), [2, n], [0, 32], [1, 2]])
                eng.tensor_tensor(wo, xi, rb, op=ALU.mult)
                eng.tensor_tensor(wo, wo, mb, op=ALU.subtract)

        xTs = {}

        def ln1_load(p):
            xT = wpool.tile([128, 32, 64], BF, tag="xT", bufs=4)
            nc.sync.dma_start_transpose(xT[:], inp_t[p])
            xTs[p] = xT

        def ln1_plane(p):
            xT = xTs.pop(p)
            # compact normalized tile: only the 64 channel slots per group
            # (no garbage upper halves), so the channel-major transpose moves
            # half the tiles; squares live in their own packed scratch
            xln = wpool.tile([128, 32, 64], BF, tag="xln", bufs=4)
            sqt = wpool.tile([128, 32, 64], BF, tag="sqt", bufs=2)
            nc.vector.tensor_tensor(sqt[:], xT[:], xT[:], op=ALU.mult)
            r2 = wpool.tile([128, 128], BF, tag="r2")
            ln_stats(xT[:], sqt[:], r2, 32)
            norm_split(xln, xT, r2[:, 0:64], r2[:, 64:128])
            # s_t partitions: 0:64 = channels of even groups, 64:128 = odd
            s_t = wpool.tile([128, 16, 128], BF, tag="scm", bufs=4)
            nc.sync.dma_start_transpose(
                s_t[:], xln[:].rearrange("p a b -> p (a b)"))
            # pad + fp8 into R[p%NS] lower: four (g-parity, h-parity) pieces
            Rt = Rs[p % NS]
            rv0 = Rt[0:64, 0:1]
            for gpar in range(2):
                for hp in range(2):
                    dst4 = BassAP(rv0.tensor,
                                  rv0.offset + (1 + 2 * gpar + hp) * PW + 1,
                                  [list(rv0.ap[0]), [4 * PW, 16], [1, 64]])
                    sv = s_t[64 * gpar:64 * gpar + 64, 0:1]
                    src4 = BassAP(sv.tensor, sv.offset + 64 * hp,
                                  [list(sv.ap[0]), [128, 16], [1, 64]])
                    nc.gpsimd.tensor_copy(dst4, src4)
            # replicate into R[(p-1)%NS] upper
            Rp = Rs[(p - 1) % NS]
            nc.sync.dma_start(Rp[64:128, 0:PLA], Rt[0:64, 0:PLA])

        def conv_plane(d):
            # two 512-position chunks share one [128, 1024] psum pair so a
            # single wide gelu evacuates both (halves the Act init overhead)
            TA = Rs[d % NS]
            TB = Rs[(d + 1) % NS]
            for cw in range(4):
                ps = psA.tile([128, 1024], F32, tag="mm128")
                for h in range(2):
                    cb = 2 * cw + h
                    pat = 0 if cb == 0 else (2 if cb == 7 else 1)
                    dcor = (REGB + REGS * pat + 1) - ((8 * cb + 1) * PW + 1)
                    psh = ps[:, h * 512:(h + 1) * 512]
                    mlist = []
                    for (T, WT, w4t) in ((TA, wa, wa4[:, d]), (TB, wb, wb4[:])):
                        bases = [8 * cb * PW, (8 * cb + 1) * PW,
                                 (8 * cb + 2) * PW, 8 * cb * PW + 1,
                                 (8 * cb + 1) * PW + 1]
                        dlts = [2, 2, 2, 132,
                                dcor if T is TA else 2]
                        for i in range(5):
                            v = T[:, bases[i]:bases[i] + 1]
                            rhs = BassAP(v.tensor, v.offset,
                                         [list(v.ap[0]), [dlts[i], 2],
                                          [PW, 8], [1, 64]])
                            lhs = WT[:, i] if i < 4 else w4t
                            mlist.append((lhs, rhs))
                    for i, (lhs, rhs) in enumerate(mlist):
                        nc.tensor.matmul(psh, lhs, rhs,
                                         start=(i == 0), stop=(i == 9),
                                         perf_mode=DR)
                col = d * 4 + cw
                nc.scalar.activation(
                    xg[:, col * 1024:(col + 1) * 1024], ps[:], AF.Gelu,
                    bias=modb2[:], scale=sd8[:],
                    accum_out=pools[:, col:col + 1])

        ln1_load(0)
        ln1_load(1)
        for p in range(NHALO):
            if p + 2 < NHALO:
                ln1_load(p + 2)
            ln1_plane(p)
            if p >= 3:
                conv_plane(p - 3)
        conv_plane(NPL - 1)

        # pass-2 consts: issued here so they ride the HWDGE idle slots of
        # pass 1 instead of delaying the first plane transposes
        w3tp = const("w3tp", [128, 64], BF)
        identA = const("identA", [65, 64], BF)
        identB = const("identB", [65, 64], BF)
        beta3 = const("beta3", [64, 1], F32)
        b3b = const("b3b", [64, 1], F32)
        w4T = const("w4T", [64, 128], BF)
        # doubled copy: matmul lhs/rhs must share a base partition, and the
        # compact x2cm puts odd-group channels on partitions 64:128
        w4T2 = cpool.tile([128, 128], BF, tag="w4T2", name="w4T2")
        nc.sync.dma_start(w4T2[0:64, :], dram["w4T"][:])
        nc.sync.dma_start(w4T2[64:128, :], dram["w4T"][:])
        b4 = const("b4", [128, 1], F32)
        w5gT = const("w5gT", [128, 64], BF)
        b5g = const("b5g", [64, 1], F32)
        scawT = const("scawT", [128, 128], BF)
        scab = const("scab", [128, 1], F32)

        if DBG:
            nc.sync.dma_start(dbg_xg[:], xg[:])
            nc.sync.dma_start(dbg_R[:], Rs[2][:])

        # ---------------- pooled -> gate ----------------
        pooled = cpool.tile([128, 1], F32, tag="pooled")
        nc.vector.tensor_reduce(pooled[:], pools[:], axis=AX.X, op=ALU.add)
        nc.sync.dma_start(cc_a[:], pooled[:])
        # AllGather + local sum: the naive collective cost model charges
        # AllReduce 1.875x the AllGather price for the same tiny payload.
        nc.gpsimd.collective_compute(
            "AllGather", ALU.bypass,
            replica_groups=[[0, 1, 2, 3], [4, 5, 6, 7]],
            ins=[cc_a[:]], outs=[cc_b[:]])
        prbuf = cpool.tile([128, 4], F32, tag="prbuf", name="prbuf")
        nc.sync.dma_start(prbuf[:], cc_b[:].rearrange("a b -> b a"))
        pooled2f = cpool.tile([128, 1], F32, tag="pooled2f", name="pooled2f")
        nc.vector.tensor_reduce(pooled2f[:], prbuf[:], axis=AX.X, op=ALU.add)
        pooled2 = cpool.tile([128, 1], BF, tag="pooled2", name="pooled2")
        nc.vector.tensor_copy(pooled2[:], pooled2f[:])
        psg = psA.tile([128, 512], F32, tag="mm128")
        nc.tensor.matmul(psg[:, 0:1], scawT[:], pooled2[:], start=True,
                         stop=True)
        gate = cpool.tile([128, 1], F32, tag="gatev")
        nc.scalar.activation(gate[:], psg[:, 0:1], AF.Identity, bias=scab[:])
        w3gb = cpool.tile([128, 64], BF, tag="w3gb")
        nc.vector.tensor_scalar_mul(w3gb[:], w3tp[:], gate[:])
        nc.vector.tensor_copy(w3g8[:], w3gb[:])

        # ---------------- PASS 2 ----------------
        def y_form(d):
            ifp = ifps[d % 2]
            y = ys[d % 4]
            for cw in range(4):
                sl = slice(cw * 1024, (cw + 1) * 1024)
                ps3 = psB.tile([64, 1024], F32, tag="mm64", bufs=2)
                for h in range(2):
                    cb = 2 * cw + h
                    hsl = slice(cb * 512, (cb + 1) * 512)
                    psh = ps3[:, h * 512:(h + 1) * 512]
                    xs = xg[:, d * HWC + cb * 512:d * HWC + (cb + 1) * 512]
                    if cw in Y_DVE:
                        nc.tensor.matmul(psh, w3g8[:], xs,
                                         start=True, stop=True)
                    else:
                        nc.tensor.matmul(psh, w3g8[:], xs,
                                         start=True, stop=False)
                        nc.tensor.matmul(psh, identA[:], ifp[0:65, hsl],
                                         start=False, stop=True)
                if cw in Y_DVE:
                    nc.vector.affine_then_add(y[0:64, sl], ps3[:],
                                              ifp[0:64, sl],
                                              scale=beta3[:], bias=b3b[:])
                else:
                    nc.scalar.activation(y[0:64, sl], ps3[:], AF.Identity,
                                         scale=beta3[:])

        # 4th y buffer carved from xg planes 0-1 (dead after y_form(0/1));
        # its ones-row is DMA'd lazily (tile orders it after the pass-1
        # gelu writes and the y_form(0/1) reads of that xg region)
        xgbf = xg[:].bitcast(BF)
        ys.append(xgbf[0:65, 0:HWC])
        nc.sync.dma_start(xgbf[64:65, 0:HWC], ones_i[:])
        x2cms = {}

        def stage_a(d):
            """y(d) -> yT -> LN2 stats/norm -> x2cm(d) (compact layout:
            x2cm partitions 0:64 = channels of even groups, 64:128 odd)"""
            y = ys[d % 4]
            yT = wpool.tile([128, 32, 64], BF, tag="xT", bufs=4)
            nc.sync.dma_start_transpose(yT[:], y[0:64, :])
            xln2 = wpool.tile([128, 32, 64], BF, tag="xln", bufs=4)
            sqt = wpool.tile([128, 32, 64], BF, tag="sqt", bufs=2)
            nc.vector.tensor_tensor(sqt[:], yT[:], yT[:], op=ALU.mult)
            r2b = wpool.tile([128, 128], BF, tag="r2b")
            ln_stats(yT[:], sqt[:], r2b, 32)
            norm_split(xln2, yT, r2b[:, 0:64], r2b[:, 64:128])
            x2cm = wpool.tile([128, 16, 128], BF, tag="scm", bufs=4)
            nc.sync.dma_start_transpose(
                x2cm[:], xln2[:].rearrange("p a b -> p (a b)"))
            x2cms[d] = x2cm

        def stage_b(d):
            """x2cm(d) -> w4 -> gelu -> w5 -> outb(d); inputs 2 planes old.

            Parity-split: the compact x2cm holds even-group channels on
            partitions 0:64 and odd-group on 64:128, so each matmul slices
            the rhs partition range and the +y / outb accesses use
            256-strided position views. Software-pipelined by one chunk so
            the in-order Act stream never waits on the w5 matmuls it fed."""
            y = ys[d % 4]
            x2cm = x2cms.pop(d)
            yv0 = y[0:64, 0:1]
            y65 = y[0:65, 0:1]
            pend_evac = None

            def flush_evac():
                nonlocal pend_evac
                if pend_evac is None:
                    return
                idx, ps5, ob_ap, y_ap = pend_evac
                if idx in O_DVE:
                    nc.vector.affine_then_add(ob_ap, ps5[:], y_ap,
                                              scale=1.0, bias=b5g[:])
                else:
                    nc.scalar.activation(ob_ap, ps5[:], AF.Identity)
                pend_evac = None

            for hf in range(2):
                outb = Rv[(2 * d + hf) % 4][0:64, 0:2048]
                ov0 = outb[:, 0:1]
                for par in range(2):
                    idx = 2 * hf + par
                    xcv = x2cm[64 * par:64 * par + 64, 0:1]
                    ps4 = psA.tile([128, 1024], F32, tag="mm128")
                    for h in range(2):
                        rhs = BassAP(xcv.tensor,
                                     xcv.offset + hf * 1024 + h * 512,
                                     [list(xcv.ap[0]), [1, 512]])
                        nc.tensor.matmul(ps4[:, h * 512:(h + 1) * 512],
                                         w4T2[64 * par:64 * par + 64, :],
                                         rhs, start=True, stop=True)
                    xg2t = wpool.tile([128, 1024], BF, tag="xg2", bufs=3)
                    xg2 = xg2t[:]
                    nc.scalar.activation(xg2, ps4[:], AF.Gelu, bias=b4[:])
                    flush_evac()
                    ps5 = psB.tile([64, 1024], F32, tag="mm64", bufs=2)
                    for h in range(2):
                        psh = ps5[:, h * 512:(h + 1) * 512]
                        yb = BassAP(y65.tensor,
                                    y65.offset + hf * 2048 + h * 1024
                                    + par * 128,
                                    [list(y65.ap[0]), [256, 4], [1, 128]])
                        if idx in O_DVE:
                            nc.tensor.matmul(psh, w5gT[:],
                                             xg2[:, h * 512:(h + 1) * 512],
                                             start=True, stop=True)
                        else:
                            nc.tensor.matmul(psh, w5gT[:],
                                             xg2[:, h * 512:(h + 1) * 512],
                                             start=True, stop=False)
                            nc.tensor.matmul(psh, identB[:], yb,
                                             start=False, stop=True)
                    ob_ap = BassAP(ov0.tensor, ov0.offset + par * 128,
                                   [list(ov0.ap[0]), [256, 8], [1, 128]])
                    y_ap = BassAP(yv0.tensor,
                                  yv0.offset + hf * 2048 + par * 128,
                                  [list(yv0.ap[0]), [256, 8], [1, 128]])
                    pend_evac = (idx, ps5, ob_ap, y_ap)
                if hf == 1:
                    flush_evac()
                pend_outs.append((d, hf, outb))

        pend_outs = []
        nc.sync.dma_start(ifps[0][0:64, :], inp_b[0])
        nc.sync.dma_start(ifps[1][0:64, :], inp_b[1])
        y_form(0)
        # stage A(d) runs 2 planes ahead of stage B(d): B's inputs are long
        # ready, so the Act/PE ffn stream never stalls on the LN2 chain
        for d in range(NPL + 2):
            if d + 2 < NPL:
                nc.sync.dma_start(ifps[(d + 2) % 2][0:64, :], inp_b[d + 2])
            if d + 1 < NPL:
                y_form(d + 1)
            if d < NPL:
                stage_a(d)
            for (pd, phf, pob) in pend_outs:
                nc.sync.dma_start(
                    out_d[pd][:, phf * 2048:(phf + 1) * 2048], pob)
            pend_outs.clear()
            if d >= 2:
                stage_b(d - 2)

        for (pd, phf, pob) in pend_outs:
            nc.sync.dma_start(
                out_d[pd][:, phf * 2048:(phf + 1) * 2048], pob)
        pend_outs.clear()

    nc.compile()
    return nc


def _host_prep(inputs):
    inp = np.asarray(inputs["inp"], np.float32)
    style = np.asarray(inputs["style_vector"], np.float32)
    w1 = np.asarray(inputs["w1"], np.float32)
    b1 = np.asarray(inputs["b1"], np.float32)
    mod_w = np.asarray(inputs["mod_w"], np.float32)
    mod_b = np.asarray(inputs["mod_b"], np.float32)
    style_w = np.asarray(inputs["style_w"], np.float32)
    style_b = np.asarray(inputs["style_b"], np.float32)
    sca_w = np.asarray(inputs["sca_w"], np.float32)
    sca_b = np.asarray(inputs["sca_b"], np.float32)
    w3 = np.asarray(inputs["w3"], np.float32)
    b3 = np.asarray(inputs["b3"], np.float32)
    w4 = np.asarray(inputs["w4"], np.float32)
    b4 = np.asarray(inputs["b4"], np.float32)
    w5 = np.asarray(inputs["w5"], np.float32)
    b5 = np.asarray(inputs["b5"], np.float32)
    ln1_w = np.asarray(inputs["ln1_w"], np.float32).reshape(C)
    ln2_w = np.asarray(inputs["ln2_w"], np.float32).reshape(C)
    beta = np.asarray(inputs["beta"], np.float32).reshape(C)
    gamma = np.asarray(inputs["gamma"], np.float32).reshape(C)

    s = style @ style_w.T + style_b
    k2 = (mod_w ** 2).sum(axis=(1, 2, 3, 4))
    demod = 1.0 / np.sqrt(k2[None] * s * s + 1e-8)
    sdv = s * demod                                    # [B, DW]

    W1t = w1 * ln1_w[None, :]                          # [DW, C]
    wdw = mod_w[:, 0]                                  # [DW, 3, 3, 3]

    # per-out-channel pow2 scale for conv fp8 weights
    wmax = (np.abs(W1t).max(axis=1) * np.abs(wdw).reshape(DW, -1).max(axis=1))
    s_exp = np.floor(np.log2(16.0 / np.maximum(wmax, 1e-12)))
    s_exp = np.clip(s_exp, -20, 20)
    wsc = (2.0 ** s_exp)                               # [DW]

    def wtap(kd, kh, kw):
        # [64, 128]: lhsT rows = in-ch, cols = out-ch, scaled
        return (W1t * wdw[:, kd, kh, kw][:, None] * wsc[:, None]).T

    j_pairs = [((0, 0), (0, 2)), ((1, 0), (1, 2)), ((2, 0), (2, 2)),
               ((0, 1), (2, 1))]
    wa = np.zeros((4, 128, 2, 128), np.float32)
    wb = np.zeros((4, 128, 2, 128), np.float32)
    for i, (t0, t1) in enumerate(j_pairs):
        for j, (kh, kw) in enumerate((t0, t1)):
            wa[i, 0:64, j, :] = wtap(0, kh, kw)
            wa[i, 64:128, j, :] = wtap(1, kh, kw)
            wb[i, 64:128, j, :] = wtap(2, kh, kw)
    wb4 = np.zeros((128, 2, 128), np.float32)
    wb4[64:128, 0, :] = wtap(2, 1, 1)

    def S(cd, ch, cw):
        vd = {0: [1, 2], 1: [0, 1, 2], 2: [0, 1]}[cd]
        vh = {0: [1, 2], 1: [0, 1, 2], 2: [0, 1]}[ch]
        vw = {0: [1, 2], 1: [0, 1, 2], 2: [0, 1]}[cw]
        return wdw[:, vd][:, :, vh][:, :, :, vw].sum(axis=(1, 2, 3))

    base_mid = S(1, 1, 1)                              # sum of all taps

    def corr_for(dcase):
        c = np.zeros((9, 128), np.float32)
        base = S(dcase, 1, 1)
        c[0] = base - base_mid
        ch0 = S(dcase, 0, 1) - base
        ch1 = S(dcase, 2, 1) - base
        cw0 = S(dcase, 1, 0) - base
        cw1 = S(dcase, 1, 2) - base
        c[1], c[2], c[3], c[4] = ch0, ch1, cw0, cw1
        c[5] = S(dcase, 0, 0) - S(dcase, 0, 1) - S(dcase, 1, 0) + base
        c[6] = S(dcase, 0, 2) - S(dcase, 0, 1) - S(dcase, 1, 2) + base
        c[7] = S(dcase, 2, 0) - S(dcase, 2, 1) - S(dcase, 1, 0) + base
        c[8] = S(dcase, 2, 2) - S(dcase, 2, 1) - S(dcase, 1, 2) + base
        return c * b1[None, :]

    corr_tab = {c: corr_for(c) for c in (0, 1, 2)}

    g = np.zeros((9, 64, 64), np.float32)
    g[0] = 1.0
    g[1, 0, :] = 1.0
    g[2, 63, :] = 1.0
    g[3, :, 0] = 1.0
    g[4, :, 63] = 1.0
    g[5, 0, 0] = 1.0
    g[6, 0, 63] = 1.0
    g[7, 63, 0] = 1.0
    g[8, 63, 63] = 1.0
    hsl = {0: slice(0, 8), 1: slice(8, 16), 2: slice(56, 64)}
    reg = np.zeros((9, 3 * REGS), np.float32)
    for pat in range(3):
        blk = g[:, hsl[pat], :]                        # [9, 8, 64]
        for r in range(8):
            o = REGS * pat + 1 + r * PW
            reg[:, o:o + 64] = blk[:, r, :]

    # w3: per-out-col scale 2^u (expect gate ~O(0.25))
    w3T = w3.T                                         # [DW, C].T = [128, 64]
    u_exp = np.floor(np.log2(16.0 / np.maximum(
        np.abs(w3T).max(axis=0) * 0.25, 1e-12)))
    u_exp = np.clip(u_exp, -20, 20)
    usc = 2.0 ** u_exp                                 # [C]
    w3tp = (w3T * usc[None, :]).astype(bf)
    beta3 = (beta / usc).reshape(64, 1).astype(np.float32)
    b3b = (b3 * beta).reshape(64, 1).astype(np.float32)
    identA = np.zeros((65, 64), np.float32)
    identA[0:64] = np.diag(usc / beta)
    identA[64] = usc * b3
    identB = np.zeros((65, 64), np.float32)
    identB[0:64] = np.eye(64)
    identB[64] = b5 * gamma

    common = dict(
        ones=np.ones((1, HWC), bf),
        wa=wa.transpose(1, 0, 2, 3).astype(f8),
        wb=wb.transpose(1, 0, 2, 3).astype(f8), wb4=wb4.astype(f8),
        reg=reg.astype(f8),
        modb2=None,  # per-core (sd-dependent)
        w3tp=w3tp,
        identA=identA.astype(bf), identB=identB.astype(bf),
        beta3=beta3, b3b=b3b,
        w4T=(w4 * ln2_w[None, :]).T.astype(bf),
        b4=b4.reshape(128, 1).astype(np.float32),
        w5gT=(w5 * gamma[:, None]).T.astype(bf),
        b5g=(b5 * gamma).reshape(64, 1).astype(np.float32),
        scawT=(sca_w.T / float(D * H * W)).astype(bf),
        scab=sca_b.reshape(128, 1).astype(np.float32),
    )

    in_maps = []
    for k in range(8):
        b, d0 = k // 4, (k % 4) * NPL
        ip = inp[b]
        halo = np.zeros((NHALO, C, HWC), np.float32)
        lo, hi = max(d0 - 1, 0), min(d0 + NPL + 1, D)
        halo[lo - (d0 - 1):hi - (d0 - 1)] = (
            ip[:, lo:hi].transpose(1, 0, 2, 3).reshape(hi - lo, C, HWC))
        wa4 = np.zeros((NPL, 128, 2, 128), np.float32)
        for i in range(NPL):
            dg = d0 + i
            dcase = 0 if dg == 0 else (2 if dg == D - 1 else 1)
            wa4[i, 0:64, 0, :] = wtap(0, 1, 1)
            wa4[i, 64:128, 0, :] = wtap(1, 1, 1)
            wa4[i, 0:9, 1, :] = corr_tab[dcase] * wsc[None, :]
        m = dict(common)
        m["inp_t"] = halo.astype(bf)
        m["inp_b"] = np.ascontiguousarray(
            ip[:, d0:d0 + NPL].transpose(1, 0, 2, 3)
            .reshape(NPL, C, HWC)).astype(bf)
        m["wa4"] = wa4.transpose(1, 0, 2, 3).astype(f8)
        m["sd8"] = (sdv[b] / wsc).reshape(128, 1).astype(np.float32)
        m["modb2"] = (mod_b + sdv[b] * base_mid * b1).reshape(
            128, 1).astype(np.float32)
        in_maps.append(m)
    return in_maps


def kernel(**inputs):
    from concourse.bass_utils import run_bass_kernel_spmd
    if "nc" not in _CACHE:
        _CACHE["nc"] = _build()
    nc = _CACHE["nc"]
    in_maps = _host_prep(inputs)
    res = run_bass_kernel_spmd(nc, in_maps, list(range(8)))
    _CACHE["last_res"] = res
    out = np.empty((2, C, D, H, W), np.float32)
    for k in range(8):
        b, d0 = k // 4, (k % 4) * NPL
        o = np.asarray(res.results[k]["out"]).astype(np.float32)
        out[b, :, d0:d0 + NPL] = o.reshape(NPL, C, H, W).transpose(1, 0, 2, 3)
    return out

